# revision 1
# baseline (speedup 1.0000x reference)
"""TRN2 Bass kernel for GPT-style causal self-attention with RoPE.

Reference (B=2, S=2048, D=1024, H=16, dk=64):
  qkv = hidden @ c_attn_w + c_attn_b; rope(q), rope(k) via position_ids;
  out = softmax(causal(q k^T / 8)) v, merged heads, @ c_proj_w + c_proj_b.

Sharding across 8 NeuronCores: core c = 4*b + g handles batch b and head
group g (4 heads = 256 dims). Each core computes its full S x S attention
and a row-sliced c_proj partial; the host sums the 4 partials per batch.

Device pipeline per core (all matmuls float32r):
  1. QKV in natural layout from host-transposed hiddenT (lhsT = hiddenT
     chunks); bias via K=1 ones-row matmul; rope applied in natural layout
     (2 DVE multiplies using a pair-swap access pattern); the rope add is
     folded into two accumulated PE transposes -> qT/kT [2-head dk, S];
     v evicted to [s, 4 x 65] layout with a ones column per head.
  2. Per head-pair, per 512-wide q chunk: scores^T via row-tiled K=64
     matmul pairs (two heads concurrently in the PE array); exp on ScalarE
     (scale=1/8); causal diagonal mask (0/1) on GPSIMD post-exp;
     PV: out[0:65] = [v | ones]^T @ P^T accumulated over k blocks (row 64 =
     softmax denominators); normalize via reciprocal_approx_fast + GPSIMD
     partition_broadcast + DVE multiply.
  3. Transposed projection projT[d, s] = Wp_chunk^T @ attn^T with c_proj_b
     added through the Identity-activation per-partition bias.
Output per core: outT [1024, 2048] partial; host sums per batch, transposes.
"""

from contextlib import ExitStack

import numpy as np

import concourse.bacc as bacc
import concourse.tile as tile
import concourse.mybir as mybir
from concourse.bass_utils import run_bass_kernel_spmd

f32 = mybir.dt.float32
f32r = mybir.dt.float32r
AF = mybir.ActivationFunctionType
ALU = mybir.AluOpType

S = 2048
D = 1024
HD = 256           # head dims per core (4 heads x 64)
SB = S // 128      # 16
KC = D // 128      # 8
NCH = S // 512     # 4


def build_attention_nc(num_devices=8):
    nc = bacc.Bacc("TRN2", target_bir_lowering=False, debug=False,
                   num_devices=num_devices)

    hT_d = nc.dram_tensor("hT", [D, S], f32r, kind="ExternalInput")
    wqkv_d = nc.dram_tensor("wqkv", [D, 768], f32r, kind="ExternalInput")
    bqkv_d = nc.dram_tensor("bqkv", [1, 768], f32r, kind="ExternalInput")
    cos4_d = nc.dram_tensor("cos4", [S, HD], f32r, kind="ExternalInput")
    sins4_d = nc.dram_tensor("sins4", [S, HD], f32r, kind="ExternalInput")
    wp_d = nc.dram_tensor("wp", [HD, D], f32r, kind="ExternalInput")
    bp_d = nc.dram_tensor("bp", [128, 8], f32, kind="ExternalInput")
    mask01_d = nc.dram_tensor("mask01", [128, 128], f32r, kind="ExternalInput")
    ones64_d = nc.dram_tensor("ones64", [128, 64], f32r, kind="ExternalInput")
    ident_d = nc.dram_tensor("ident", [128, 128], f32r, kind="ExternalInput")
    onesrow_d = nc.dram_tensor("ones_row", [1, 128], f32r, kind="ExternalInput")
    outT_d = nc.dram_tensor("outT", [D, S], f32, kind="ExternalOutput")

    with tile.TileContext(nc) as tc, ExitStack() as top:
        const = top.enter_context(tc.tile_pool(name="const", bufs=1))
        ident = const.tile([128, 128], f32r, tag="ident")
        nc.sync.dma_start(ident[:], ident_d.ap())
        mask01 = const.tile([128, 128], f32r, tag="mask01")
        nc.sync.dma_start(mask01[:], mask01_d.ap())
        ones_row = const.tile([1, 128], f32r, tag="ones_row")
        nc.sync.dma_start(ones_row[:], onesrow_d.ap())
        bp_sb = const.tile([128, 8], f32, tag="bp")
        nc.sync.dma_start(bp_sb[:], bp_d.ap())

        persist = top.enter_context(tc.tile_pool(name="persist", bufs=1))
        qT = [persist.tile([128, S], f32r, tag=f"qT{hp}", name=f"qT{hp}")
              for hp in range(2)]
        kT = [persist.tile([128, S], f32r, tag=f"kT{hp}", name=f"kT{hp}")
              for hp in range(2)]
        v_sb = persist.tile([128, SB, 4, 65], f32r, tag="v")
        ones64 = const.tile([128, 64], f32r, tag="ones64")
        nc.sync.dma_start(ones64[:], ones64_d.ap())
        nc.scalar.copy(v_sb[:, :, :, 64],
                       ones64[:].rearrange("p (a b) -> p a b", a=SB))
        wp_sb = persist.tile([128, 2, D], f32r, tag="wp")
        for kc2 in range(2):
            nc.sync.dma_start(wp_sb[:, kc2, :],
                              wp_d.ap()[kc2 * 128:(kc2 + 1) * 128, :])

        # ============ stage 1: QKV + rope + transpose ============
        with ExitStack() as st1:
            hT_pool = st1.enter_context(tc.tile_pool(name="hT", bufs=1))
            w_pool = st1.enter_context(tc.tile_pool(name="w", bufs=1))
            trig_pool = st1.enter_context(tc.tile_pool(name="trig", bufs=2))
            qkv_ps = st1.enter_context(
                tc.tile_pool(name="qkv_ps", bufs=2, space="PSUM"))
            tr_ps = st1.enter_context(
                tc.tile_pool(name="tr_ps", bufs=2, space="PSUM"))
            rope_pool = st1.enter_context(tc.tile_pool(name="rope", bufs=1))

            hT_sb = [hT_pool.tile([128, S], f32r, tag=f"hT{kc}", name=f"hT{kc}")
                     for kc in range(KC)]
            for kc in range(KC):
                nc.sync.dma_start(hT_sb[kc][:],
                                  hT_d.ap()[kc * 128:(kc + 1) * 128, :])
            w_sb = [w_pool.tile([128, 768], f32r, tag=f"w{kc}", name=f"w{kc}")
                    for kc in range(KC)]
            for kc in range(KC):
                nc.sync.dma_start(w_sb[kc][:],
                                  wqkv_d.ap()[kc * 128:(kc + 1) * 128, :])
            bqkv_sb = w_pool.tile([1, 768], f32r, tag="bqkv")
            nc.sync.dma_start(bqkv_sb[:], bqkv_d.ap())

            for sg in range(SB // 4):
                rope_tiles = {}
                for sbl in range(4):
                    sb = sg * 4 + sbl
                    cos_t = trig_pool.tile([128, HD], f32r, tag=f"cos{sbl}",
                                           name=f"cos{sbl}")
                    sins_t = trig_pool.tile([128, HD], f32r, tag=f"sin{sbl}",
                                            name=f"sin{sbl}")
                    nc.sync.dma_start(
                        cos_t[:], cos4_d.ap()[sb * 128:(sb + 1) * 128, :])
                    nc.sync.dma_start(
                        sins_t[:], sins4_d.ap()[sb * 128:(sb + 1) * 128, :])

                    qkv_p = qkv_ps.tile([128, 768], f32, tag="qkv_p")
                    for kc in range(KC):
                        lhsT = hT_sb[kc][:, sb * 128:(sb + 1) * 128]
                        nc.tensor.matmul(qkv_p[:, 0:512], lhsT,
                                         w_sb[kc][:, 0:512],
                                         start=(kc == 0), stop=False)
                        nc.tensor.matmul(qkv_p[:, 512:768], lhsT,
                                         w_sb[kc][:, 512:768],
                                         start=(kc == 0), stop=False)
                    nc.tensor.matmul(qkv_p[:, 0:512], ones_row[:],
                                     bqkv_sb[:, 0:512], start=False, stop=True)
                    nc.tensor.matmul(qkv_p[:, 512:768], ones_row[:],
                                     bqkv_sb[:, 512:768], start=False,
                                     stop=True)

                    for qk in range(2):
                        base = qk * HD
                        pin = qkv_p[:, base:base + HD]
                        pin_sw = qkv_p[:, base:base + HD].rearrange(
                            "p (h t d) -> p h t d", h=4, t=2)[:, :, ::-1, :]
                        t1 = rope_pool.tile([128, HD], f32r,
                                            tag=f"t1_{qk}_{sbl}",
                                            name=f"t1_{qk}_{sbl}")
                        t2 = rope_pool.tile([128, HD], f32r,
                                            tag=f"t2_{qk}_{sbl}",
                                            name=f"t2_{qk}_{sbl}")
                        nc.vector.tensor_tensor(t1[:], pin, cos_t[:],
                                                op=ALU.mult)
                        nc.vector.tensor_tensor(
                            t2[:].rearrange("p (h t d) -> p h t d", h=4, t=2),
                            pin_sw,
                            sins_t[:].rearrange("p (h t d) -> p h t d",
                                                h=4, t=2),
                            op=ALU.mult)
                        rope_tiles[(qk, sbl)] = (t1, t2)

                    nc.scalar.copy(
                        v_sb[:, sb, :, 0:64],
                        qkv_p[:, 512:768].rearrange("p (h d) -> p h d", h=4))

                for qk in range(2):
                    dest = qT if qk == 0 else kT
                    for hp in range(2):
                        tp = tr_ps.tile([128, 512], f32, tag="tr_p")
                        for sbl in range(4):
                            t1, t2 = rope_tiles[(qk, sbl)]
                            dst = tp[:, sbl * 128:(sbl + 1) * 128].bitcast(f32r)
                            nc.tensor.matmul(
                                dst, t1[:, hp * 128:(hp + 1) * 128], ident[:],
                                is_transpose=True, start=True, stop=False)
                            nc.tensor.matmul(
                                dst, t2[:, hp * 128:(hp + 1) * 128], ident[:],
                                is_transpose=True, start=False, stop=True)
                        nc.any.tensor_copy(
                            dest[hp][:, sg * 512:(sg + 1) * 512], tp[:])

        # ============ stages 2+3 ============
        st23 = top.enter_context(ExitStack())
        a_pool = st23.enter_context(tc.tile_pool(name="a_pool", bufs=1))
        aT2 = [a_pool.tile([128, S], f32r, tag=f"aT2{hp}", name=f"aT2{hp}")
               for hp in range(2)]
        aTo = [a_pool.tile([64, S], f32r, tag=f"aTo{hp}", name=f"aTo{hp}")
               for hp in range(2)]
        # ============ stage 2: attention ============
        with ExitStack() as st2:
            pt_pool = st2.enter_context(tc.tile_pool(name="pt", bufs=17))
            st_ps = st2.enter_context(
                tc.tile_pool(name="st_ps", bufs=2, space="PSUM"))
            out_ps = st2.enter_context(
                tc.tile_pool(name="out_ps", bufs=2, space="PSUM"))
            nrm_pool = st2.enter_context(tc.tile_pool(name="nrm", bufs=3))

            for hp in range(2):
                for c in range(NCH):
                    nkb = 4 * c + 4
                    pts = []
                    for kb in range(nkb):
                        q0 = max(512 * c, 128 * kb)
                        off = q0 - 512 * c
                        st_p = st_ps.tile([128, 2, 512], f32, tag="st_p")
                        for h2 in range(2):
                            nc.tensor.matmul(
                                st_p[:, h2, off:512],
                                kT[hp][h2 * 64:(h2 + 1) * 64,
                                       kb * 128:(kb + 1) * 128],
                                qT[hp][h2 * 64:(h2 + 1) * 64,
                                       q0:512 * (c + 1)],
                                start=True, stop=True,
                                tile_position=(h2 * 64, 0))
                        pt = pt_pool.tile([128, 2, 512], f32r, tag="pt")
                        nc.scalar.activation(pt[:, :, off:512],
                                             st_p[:, :, off:512],
                                             AF.Exp, scale=0.125)
                        if 128 * kb >= 512 * c:
                            for h2 in range(2):
                                nc.gpsimd.tensor_mul(
                                    pt[:, h2, off:off + 128],
                                    pt[:, h2, off:off + 128],
                                    mask01[:])
                        pts.append((kb, off, pt))

                    for h2 in range(2):
                        h = 2 * hp + h2
                        o_p = out_ps.tile([128, 512], f32, tag="o_p")
                        for (kb, off, pt) in pts:
                            nc.tensor.matmul(
                                o_p[0:65, off:512],
                                v_sb[:, kb, h, :],
                                pt[:, h2, off:512],
                                start=(kb == 0), stop=(kb == nkb - 1))
                        den = nrm_pool.tile([65, 512], f32, tag="den")
                        den0 = nrm_pool.tile([1, 512], f32, tag="den0")
                        rcp0 = nrm_pool.tile([1, 512], f32, tag="rcp0")
                        bc = nrm_pool.tile([64, 512], f32, tag="bc")
                        nc.scalar.copy(den[64:65, :], o_p[64:65, :])
                        # custom-DVE recip and partition_broadcast need
                        # partition-0 operands; DMA does the cross-partition hop
                        nc.sync.dma_start(den0[:], den[64:65, :])
                        nc.vector.reciprocal_approx_fast(rcp0[:], den0[:])
                        nc.gpsimd.partition_broadcast(bc[:], rcp0[:])
                        if h2 == 0:
                            out_ap = aT2[hp][0:64, c * 512:(c + 1) * 512]
                        else:
                            out_ap = aTo[hp][0:64, c * 512:(c + 1) * 512]
                        nc.vector.tensor_tensor(out_ap, o_p[0:64, :], bc[:],
                                                op=ALU.mult)

            for hp in range(2):
                nc.sync.dma_start(aT2[hp][64:128, :], aTo[hp][:])

        # ============ stage 3: projection ============
        with ExitStack() as st3:
            pj_ps = st3.enter_context(
                tc.tile_pool(name="pj_ps", bufs=2, space="PSUM"))
            pj_sb = st3.enter_context(tc.tile_pool(name="pj_sb", bufs=3))
            for dd in range(8):
                for sc in range(NCH):
                    pp = pj_ps.tile([128, 512], f32, tag="pp")
                    for kc2 in range(2):
                        nc.tensor.matmul(
                            pp[:],
                            wp_sb[:, kc2, dd * 128:(dd + 1) * 128],
                            aT2[kc2][:, sc * 512:(sc + 1) * 512],
                            start=(kc2 == 0), stop=(kc2 == 1))
                    po = pj_sb.tile([128, 512], f32, tag="po")
                    nc.scalar.activation(po[:], pp[:], AF.Identity,
                                         bias=bp_sb[:, dd:dd + 1])
                    nc.sync.dma_start(
                        outT_d.ap()[dd * 128:(dd + 1) * 128,
                                    sc * 512:(sc + 1) * 512],
                        po[:])

    nc.finalize()
    return nc


def make_core_inputs(inputs, core):
    """Host-side shard prep for one core."""
    b, g = core // 4, core % 4
    hidden = np.asarray(inputs["hidden_states"], dtype=np.float32)
    pos = np.asarray(inputs["position_ids"])
    caw = np.asarray(inputs["c_attn_w"], dtype=np.float32)
    cab = np.asarray(inputs["c_attn_b"], dtype=np.float32)
    cpw = np.asarray(inputs["c_proj_w"], dtype=np.float32)
    cpb = np.asarray(inputs["c_proj_b"], dtype=np.float32)

    cs = slice(g * HD, (g + 1) * HD)
    wqkv = np.concatenate(
        [caw[:, cs], caw[:, D + g * HD:D + (g + 1) * HD],
         caw[:, 2 * D + g * HD:2 * D + (g + 1) * HD]], axis=1)
    bqkv = np.concatenate(
        [cab[cs], cab[D + g * HD:D + (g + 1) * HD],
         cab[2 * D + g * HD:2 * D + (g + 1) * HD]])[None, :]

    inv_freq = (1.0 / (10000.0 **
                       (np.arange(0, 64, 2, dtype=np.float64) / 64.0)))
    freqs = pos[b].astype(np.float64)[:, None] * inv_freq[None, :]
    emb = np.concatenate([freqs, freqs], axis=1)
    cos = np.cos(emb).astype(np.float32)
    sin = np.sin(emb).astype(np.float32)
    sins = sin.copy()
    sins[:, :32] *= -1.0
    cos4 = np.tile(cos, (1, 4)).astype(np.float32)
    sins4 = np.tile(sins, (1, 4)).astype(np.float32)

    bp = (cpb if g == 0 else np.zeros_like(cpb)).reshape(8, 128).T.copy()

    r = np.arange(128)
    mask01 = (r[None, :] >= r[:, None]).astype(np.float32)

    return {
        "hT": np.ascontiguousarray(hidden[b].T),
        "wqkv": np.ascontiguousarray(wqkv),
        "bqkv": np.ascontiguousarray(bqkv),
        "cos4": cos4,
        "sins4": sins4,
        "wp": np.ascontiguousarray(cpw[cs, :]),
        "bp": np.ascontiguousarray(bp.astype(np.float32)),
        "mask01": mask01,
        "ones64": np.ones((128, 64), np.float32),
        "ident": np.eye(128, dtype=np.float32),
        "ones_row": np.ones((1, 128), np.float32),
    }


_NC_CACHE = {}


def run(inputs, trace=False, **spmd_kwargs):
    """Shard, execute on 8 cores, unshard. Returns (output, BassKernelResults)."""
    if "nc" not in _NC_CACHE:
        _NC_CACHE["nc"] = build_attention_nc(num_devices=8)
    nc = _NC_CACHE["nc"]
    in_maps = [make_core_inputs(inputs, c) for c in range(8)]
    res = run_bass_kernel_spmd(nc, in_maps, core_ids=list(range(8)),
                               trace=trace, **spmd_kwargs)
    outs = []
    for b in range(2):
        acc = np.zeros((D, S), np.float64)
        for g in range(4):
            acc += res.results[b * 4 + g]["outT"].astype(np.float64)
        outs.append(acc.T.astype(np.float32))
    return np.stack(outs, axis=0), res


def kernel(**inputs) -> np.ndarray:
    out, _ = run(inputs, trace=False)
    return out



# revision 4
# speedup vs baseline: 1.0902x; 1.0902x over previous
"""TRN2 Bass kernel for GPT-style causal self-attention with RoPE (bf16).

Reference (B=2, S=2048, D=1024, H=16, dk=64):
  qkv = hidden @ c_attn_w + c_attn_b; rope(q), rope(k) via position_ids;
  out = softmax(causal(q k^T / 8)) v, merged heads, @ c_proj_w + c_proj_b.

Sharding across 8 NeuronCores: core c = 4*b + g handles batch b and head
group g (4 heads = 256 dims). Each core computes its full S x S attention
for its heads and a row-sliced c_proj partial; the host sums the 4
partials per batch.

Device pipeline per core (all matmuls bf16, fp32 PSUM accumulate):
  1. QKV weight-stationary: qkvT[do, s] = Wqkv_chunk^T @ hT directly in
     transposed layout (no PE transposes for q/k). Rope applied in the
     transposed layout: partition-swap via 4 SBUF-SBUF DMAs + 3 DVE ops
     against host-precomputed cosT/sinT tables. V transposed back to
     natural [s, d] via PE transposes with a ones column appended.
  2. Per head-pair, per 512-wide q chunk: scores^T via K=64 matmul pairs
     (two heads in PE quadrants); exp on ScalarE (scale=1/8); causal
     diagonal mask (0/1) on GPSIMD post-exp; PV accumulates [v|1]^T P^T
     (row 64 = softmax denominators). Normalize: evict PSUM to SBUF
     immediately (frees the bank), then recip + partition_broadcast +
     DVE multiply off the critical path.
  3. Projection interleaved per 512-q chunk: projT = Wp^T @ attnT, bias
     via Identity-activation, bf16 DMA out.
Output per core: outT [1024, 2048] bf16 partial; host sums per batch.
"""

from contextlib import ExitStack

import numpy as np
import ml_dtypes

import concourse.bacc as bacc
import concourse.tile as tile
import concourse.mybir as mybir
from concourse.bass_utils import run_bass_kernel_spmd

f32 = mybir.dt.float32
bf16 = mybir.dt.bfloat16
AF = mybir.ActivationFunctionType
ALU = mybir.AluOpType

S = 2048
D = 1024
HD = 256           # head dims per core (4 heads x 64)
SB = S // 128      # 16
KC = D // 128      # 8
NCH = S // 512     # 4
BF = ml_dtypes.bfloat16


def build_attention_nc(with_bias=False, num_devices=8):
    nc = bacc.Bacc("TRN2", target_bir_lowering=False, debug=False,
                   num_devices=num_devices)

    hT_d = nc.dram_tensor("hT", [D, S], bf16, kind="ExternalInput")
    wqkv_d = nc.dram_tensor("wqkv", [D, 768], bf16, kind="ExternalInput")
    cosT_d = nc.dram_tensor("cosT", [128, S], bf16, kind="ExternalInput")
    sinT_d = nc.dram_tensor("sinT", [128, S], bf16, kind="ExternalInput")
    wp_d = nc.dram_tensor("wp", [HD, D], bf16, kind="ExternalInput")
    bp_d = nc.dram_tensor("bp", [128, 8], f32, kind="ExternalInput")
    mask01_d = nc.dram_tensor("mask01", [128, 128], bf16, kind="ExternalInput")
    ones64_d = nc.dram_tensor("ones64", [128, 64], bf16, kind="ExternalInput")
    ident_d = nc.dram_tensor("ident", [128, 128], bf16, kind="ExternalInput")
    if with_bias:
        bqkv_d = nc.dram_tensor("bqkv", [1, 768], bf16, kind="ExternalInput")
        onesrow_d = nc.dram_tensor("ones_row", [1, 512], bf16,
                                   kind="ExternalInput")
    outT_d = nc.dram_tensor("outT", [D, S], bf16, kind="ExternalOutput")

    with tile.TileContext(nc) as tc, ExitStack() as top:
        const = top.enter_context(tc.tile_pool(name="const", bufs=1))
        ident = const.tile([128, 128], bf16, tag="ident")
        nc.sync.dma_start(ident[:], ident_d.ap())
        mask01 = const.tile([128, 128], bf16, tag="mask01")
        nc.sync.dma_start(mask01[:], mask01_d.ap())
        bp_sb = const.tile([128, 8], f32, tag="bp")
        nc.sync.dma_start(bp_sb[:], bp_d.ap())
        if with_bias:
            bqkv_sb = const.tile([1, 768], bf16, tag="bqkv")
            nc.sync.dma_start(bqkv_sb[:], bqkv_d.ap())
            ones_row = const.tile([1, 512], bf16, tag="ones_row")
            nc.sync.dma_start(ones_row[:], onesrow_d.ap())

        persist = top.enter_context(tc.tile_pool(name="persist", bufs=1))
        qT = [persist.tile([128, S], bf16, tag=f"qT{hp}", name=f"qT{hp}")
              for hp in range(2)]
        kT = [persist.tile([128, S], bf16, tag=f"kT{hp}", name=f"kT{hp}")
              for hp in range(2)]
        v_sb = persist.tile([128, SB, 4, 65], bf16, tag="v")
        ones64 = const.tile([128, 64], bf16, tag="ones64")
        nc.sync.dma_start(ones64[:], ones64_d.ap())
        nc.scalar.copy(v_sb[:, :, :, 64],
                       ones64[:].rearrange("p (a b) -> p a b", a=SB))
        wp_sb = persist.tile([128, 2, D], bf16, tag="wp")
        for kc2 in range(2):
            nc.scalar.dma_start(wp_sb[:, kc2, :],
                                wp_d.ap()[kc2 * 128:(kc2 + 1) * 128, :])
        attnT = [persist.tile([128, S], bf16, tag=f"attnT{hp}",
                              name=f"attnT{hp}") for hp in range(2)]
        cosT = persist.tile([128, S], bf16, tag="cosT")
        sinT = persist.tile([128, S], bf16, tag="sinT")
        for sg in range(4):
            sl = slice(sg * 512, (sg + 1) * 512)
            nc.scalar.dma_start(cosT[:, sl], cosT_d.ap()[:, sl])
            nc.scalar.dma_start(sinT[:, sl], sinT_d.ap()[:, sl])

        # ============ stage 1: QKV + rope (transposed layout) ============
        with ExitStack() as st1, nc.named_scope("qkv"):
            hT_pool = st1.enter_context(tc.tile_pool(name="hT", bufs=1))
            w_pool = st1.enter_context(tc.tile_pool(name="w", bufs=1))
            vT_pool = st1.enter_context(tc.tile_pool(name="vT", bufs=1))
            qkv_ps = st1.enter_context(
                tc.tile_pool(name="qkv_ps", bufs=3, space="PSUM"))
            tr_ps = st1.enter_context(
                tc.tile_pool(name="tr_ps", bufs=2, space="PSUM"))
            rope_pool = st1.enter_context(tc.tile_pool(name="rope", bufs=3))

            w_sb = [w_pool.tile([128, 768], bf16, tag=f"w{kc}", name=f"w{kc}")
                    for kc in range(KC)]
            for kc in range(KC):
                nc.sync.dma_start(w_sb[kc][:],
                                  wqkv_d.ap()[kc * 128:(kc + 1) * 128, :])
            hT_sb = [hT_pool.tile([128, S], bf16, tag=f"hT{kc}",
                                  name=f"hT{kc}") for kc in range(KC)]
            for sblk in range(NCH):
                sl = slice(sblk * 512, (sblk + 1) * 512)
                for kc in range(KC):
                    nc.sync.dma_start(hT_sb[kc][:, sl],
                                      hT_d.ap()[kc * 128:(kc + 1) * 128, sl])

            vT_sb = [vT_pool.tile([128, S], bf16, tag=f"vT{t}", name=f"vT{t}")
                     for t in range(2)]

            # v chunks first (do 4,5), then q/k: q hp0=0, k hp0=2, q hp1=1,
            # k hp1=3
            for do in (4, 5, 0, 2, 1, 3):
                for sblk in range(NCH):
                    sl = slice(sblk * 512, (sblk + 1) * 512)
                    qkv_p = qkv_ps.tile([128, 512], f32, tag="qkv_p")
                    for kc in range(KC):
                        nc.tensor.matmul(
                            qkv_p[:], w_sb[kc][:, do * 128:(do + 1) * 128],
                            hT_sb[kc][:, sl], start=(kc == 0),
                            stop=(kc == KC - 1 and not with_bias))
                    if with_bias:
                        nc.tensor.matmul(
                            qkv_p[:],
                            bqkv_sb[:, do * 128:(do + 1) * 128],
                            ones_row[:], start=False, stop=True)
                    if do >= 4:
                        nc.scalar.copy(vT_sb[do - 4][:, sl], qkv_p[:])
                    else:
                        dest = (qT if do in (0, 1) else kT)[do % 2]
                        qraw = rope_pool.tile([128, 512], bf16, tag="qraw")
                        nc.scalar.copy(qraw[:], qkv_p[:])
                        qsw = rope_pool.tile([128, 512], bf16, tag="qsw")
                        for blk in range(4):
                            src = (blk * 32 + 32) % 64 + 64 * (blk // 2)
                            nc.gpsimd.dma_start(
                                qsw[blk * 32:blk * 32 + 32, :],
                                qraw[src:src + 32, :])
                        qcos = rope_pool.tile([128, 512], bf16, tag="qcos")
                        nc.vector.tensor_tensor(qcos[:], qkv_p[:],
                                                cosT[:, sl], op=ALU.mult)
                        qsin = rope_pool.tile([128, 512], bf16, tag="qsin")
                        nc.vector.tensor_tensor(qsin[:], qsw[:],
                                                sinT[:, sl], op=ALU.mult)
                        nc.vector.tensor_tensor(dest[:, sl], qcos[:],
                                                qsin[:], op=ALU.add)
                # after v chunks: transpose vT -> v natural layout
                if do == 5:
                    for t in range(2):
                        for sb in range(SB):
                            tp = tr_ps.tile([128, 128], bf16, tag="tp")
                            nc.tensor.matmul(
                                tp[:], vT_sb[t][:, sb * 128:(sb + 1) * 128],
                                ident[:], is_transpose=True,
                                start=True, stop=True)
                            nc.vector.tensor_copy(
                                v_sb[:, sb, 2 * t:2 * t + 2, 0:64],
                                tp[:].rearrange("p (h d) -> p h d", h=2))

        # ============ stages 2+3 interleaved ============
        st23 = top.enter_context(ExitStack())
        st_ps = st23.enter_context(
            tc.tile_pool(name="st_ps", bufs=2, space="PSUM"))
        out_ps = st23.enter_context(
            tc.tile_pool(name="out_ps", bufs=2, space="PSUM"))
        pj_ps = st23.enter_context(
            tc.tile_pool(name="pj_ps", bufs=2, space="PSUM"))
        pt_pool = st23.enter_context(tc.tile_pool(name="pt", bufs=17))
        u_pool = st23.enter_context(tc.tile_pool(name="u", bufs=3))
        nrm_pool = st23.enter_context(tc.tile_pool(name="nrm", bufs=3))
        pj_sb = st23.enter_context(tc.tile_pool(name="pj_sb", bufs=3))

        def attn_chunk(c, hp):
            nkb = 4 * c + 4
            pts = []
            for kb in range(nkb):
                q0 = max(512 * c, 128 * kb)
                off = q0 - 512 * c
                st_p = st_ps.tile([128, 2, 512], f32, tag="st_p")
                for h2 in range(2):
                    nc.tensor.matmul(
                        st_p[:, h2, off:512],
                        kT[hp][h2 * 64:(h2 + 1) * 64,
                               kb * 128:(kb + 1) * 128],
                        qT[hp][h2 * 64:(h2 + 1) * 64, q0:512 * (c + 1)],
                        start=True, stop=True, tile_position=(h2 * 64, 0))
                pt = pt_pool.tile([128, 2, 512], bf16, tag="pt")
                nc.scalar.activation(pt[:, :, off:512], st_p[:, :, off:512],
                                     AF.Exp, scale=0.125)
                if 128 * kb >= 512 * c:
                    for h2 in range(2):
                        nc.gpsimd.tensor_mul(pt[:, h2, off:off + 128],
                                             pt[:, h2, off:off + 128],
                                             mask01[:])
                pts.append((kb, off, pt))

            for h2 in range(2):
                h = 2 * hp + h2
                o_p = out_ps.tile([128, 512], f32, tag="o_p")
                for (kb, off, pt) in pts:
                    nc.tensor.matmul(
                        o_p[0:65, off:512], v_sb[:, kb, h, :],
                        pt[:, h2, off:512],
                        start=(kb == 0), stop=(kb == nkb - 1))
                u = u_pool.tile([65, 512], f32, tag="u")
                nc.scalar.copy(u[:], o_p[0:65, :])
                den0 = nrm_pool.tile([1, 512], f32, tag="den0")
                nc.gpsimd.dma_start(den0[:], u[64:65, :])
                rcp0 = nrm_pool.tile([1, 512], f32, tag="rcp0")
                nc.vector.reciprocal_approx_fast(rcp0[:], den0[:])
                bc = nrm_pool.tile([64, 512], f32, tag="bc")
                nc.gpsimd.partition_broadcast(bc[:], rcp0[:])
                csl = slice(c * 512, (c + 1) * 512)
                if h2 == 0:
                    nc.vector.tensor_tensor(attnT[hp][0:64, csl],
                                            u[0:64, :], bc[:], op=ALU.mult)
                else:
                    aTo = u_pool.tile([64, 512], bf16, tag="aTo")
                    nc.vector.tensor_tensor(aTo[:], u[0:64, :], bc[:],
                                            op=ALU.mult)
                    nc.gpsimd.dma_start(attnT[hp][64:128, csl], aTo[:])

        def proj_chunk(c):
            csl = slice(c * 512, (c + 1) * 512)
            for dd in range(8):
                pp = pj_ps.tile([128, 512], f32, tag="pp")
                for kc2 in range(2):
                    nc.tensor.matmul(
                        pp[:], wp_sb[:, kc2, dd * 128:(dd + 1) * 128],
                        attnT[kc2][:, csl],
                        start=(kc2 == 0), stop=(kc2 == 1))
                po = pj_sb.tile([128, 512], bf16, tag="po")
                nc.scalar.activation(po[:], pp[:], AF.Identity,
                                     bias=bp_sb[:, dd:dd + 1])
                nc.sync.dma_start(
                    outT_d.ap()[dd * 128:(dd + 1) * 128, csl], po[:])

        with nc.named_scope("attn"):
            attn_chunk(0, 0)
            attn_chunk(0, 1)
            attn_chunk(1, 0)
            proj_chunk(0)
            attn_chunk(1, 1)
            attn_chunk(2, 0)
            proj_chunk(1)
            attn_chunk(2, 1)
            attn_chunk(3, 0)
            proj_chunk(2)
            attn_chunk(3, 1)
            proj_chunk(3)

    nc.finalize()
    return nc


def make_core_inputs(inputs, core, with_bias, _cache={}):
    """Host-side shard prep for one core."""
    b, g = core // 4, core % 4
    key = id(inputs)
    if _cache.get("key") != key:
        _cache.clear()
        _cache["key"] = key

    if ("hT", b) not in _cache:
        hidden = np.asarray(inputs["hidden_states"], dtype=np.float32)
        _cache[("hT", b)] = np.ascontiguousarray(hidden[b].T).astype(BF)
    if ("trig", b) not in _cache:
        pos = np.asarray(inputs["position_ids"])
        inv_freq = (1.0 / (10000.0 **
                           (np.arange(0, 64, 2, dtype=np.float64) / 64.0)))
        # pattern[d, s] = pos[s] * invf[d % 32] over d in [0, 64)
        freqsT = inv_freq[:, None] * pos[b].astype(np.float64)[None, :]
        embT = np.concatenate([freqsT, freqsT], axis=0)     # [64, S]
        cosp = np.cos(embT)
        sinp = np.sin(embT)
        sinp[:32, :] *= -1.0
        _cache[("trig", b)] = (np.tile(cosp, (2, 1)).astype(BF),
                               np.tile(sinp, (2, 1)).astype(BF))

    caw = np.asarray(inputs["c_attn_w"], dtype=np.float32)
    cab = np.asarray(inputs["c_attn_b"], dtype=np.float32)
    cpw = np.asarray(inputs["c_proj_w"], dtype=np.float32)
    cpb = np.asarray(inputs["c_proj_b"], dtype=np.float32)

    cs = slice(g * HD, (g + 1) * HD)
    wqkv = np.concatenate(
        [caw[:, cs], caw[:, D + g * HD:D + (g + 1) * HD],
         caw[:, 2 * D + g * HD:2 * D + (g + 1) * HD]], axis=1)

    bp = (cpb if g == 0 else np.zeros_like(cpb)).reshape(8, 128).T.copy()

    r = np.arange(128)
    mask01 = (r[None, :] >= r[:, None]).astype(BF)
    cosT, sinT = _cache[("trig", b)]

    out = {
        "hT": _cache[("hT", b)],
        "wqkv": np.ascontiguousarray(wqkv).astype(BF),
        "cosT": cosT,
        "sinT": sinT,
        "wp": np.ascontiguousarray(cpw[cs, :]).astype(BF),
        "bp": np.ascontiguousarray(bp.astype(np.float32)),
        "mask01": mask01,
        "ones64": np.ones((128, 64), BF),
        "ident": np.eye(128).astype(BF),
    }
    if with_bias:
        bqkv = np.concatenate(
            [cab[cs], cab[D + g * HD:D + (g + 1) * HD],
             cab[2 * D + g * HD:2 * D + (g + 1) * HD]])[None, :]
        out["bqkv"] = bqkv.astype(BF)
        out["ones_row"] = np.ones((1, 512), BF)
    return out


_NC_CACHE = {}


def run(inputs, trace=False, **spmd_kwargs):
    """Shard, execute on 8 cores, unshard. Returns (output, BassKernelResults)."""
    with_bias = bool(np.any(np.asarray(inputs["c_attn_b"])))
    if with_bias not in _NC_CACHE:
        _NC_CACHE[with_bias] = build_attention_nc(with_bias=with_bias,
                                                  num_devices=8)
    nc = _NC_CACHE[with_bias]
    in_maps = [make_core_inputs(inputs, c, with_bias) for c in range(8)]
    res = run_bass_kernel_spmd(nc, in_maps, core_ids=list(range(8)),
                               trace=trace, **spmd_kwargs)
    outs = []
    for b in range(2):
        acc = np.zeros((D, S), np.float32)
        for g in range(4):
            acc += res.results[b * 4 + g]["outT"].astype(np.float32)
        outs.append(acc.T)
    return np.stack(outs, axis=0), res


def kernel(**inputs) -> np.ndarray:
    out, _ = run(inputs, trace=False)
    return out


# revision 5
# speedup vs baseline: 1.1479x; 1.0530x over previous
"""TRN2 Bass kernel for GPT-style causal self-attention with RoPE (bf16).

Reference (B=2, S=2048, D=1024, H=16, dk=64):
  qkv = hidden @ c_attn_w + c_attn_b; rope(q), rope(k) via position_ids;
  out = softmax(causal(q k^T / 8)) v, merged heads, @ c_proj_w + c_proj_b.

Sharding across 8 NeuronCores: core c = 4*b + g handles batch b and head
group g (4 heads = 256 dims). Each core computes its full S x S attention
for its heads and a row-sliced c_proj partial; the host sums the 4
partials per batch.

Device pipeline per core (all matmuls bf16, fp32 PSUM accumulate):
  1. QKV weight-stationary: qkvT[do, s] = Wqkv_chunk^T @ hT directly in
     transposed layout (no PE transposes for q/k). Rope applied in the
     transposed layout: partition-swap via 4 SBUF-SBUF DMAs + 3 DVE ops
     against host-precomputed cosT/sinT tables. V transposed back to
     natural [s, d] via PE transposes with a ones column appended.
  2. Per head-pair, per 512-wide q chunk: scores^T via K=64 matmul pairs
     (two heads in PE quadrants); exp on ScalarE (scale=1/8); causal
     diagonal mask (0/1) on GPSIMD post-exp; PV accumulates [v|1]^T P^T
     (row 64 = softmax denominators). Normalize: evict PSUM to SBUF
     immediately (frees the bank), then recip + partition_broadcast +
     DVE multiply off the critical path.
  3. Projection interleaved per 512-q chunk: projT = Wp^T @ attnT, bias
     via Identity-activation, bf16 DMA out.
Output per core: outT [1024, 2048] bf16 partial; host sums per batch.
"""

from contextlib import ExitStack

import numpy as np
import ml_dtypes

import concourse.bacc as bacc
import concourse.tile as tile
import concourse.mybir as mybir
from concourse.bass_utils import run_bass_kernel_spmd

f32 = mybir.dt.float32
bf16 = mybir.dt.bfloat16
AF = mybir.ActivationFunctionType
ALU = mybir.AluOpType

S = 2048
D = 1024
HD = 256           # head dims per core (4 heads x 64)
SB = S // 128      # 16
KC = D // 128      # 8
NCH = S // 512     # 4
BF = ml_dtypes.bfloat16


def build_attention_nc(with_bias=False, num_devices=8):
    nc = bacc.Bacc("TRN2", target_bir_lowering=False, debug=False,
                   num_devices=num_devices)

    hT_d = nc.dram_tensor("hT", [D, S], bf16, kind="ExternalInput")
    wqkv_d = nc.dram_tensor("wqkv", [D, 768], bf16, kind="ExternalInput")
    cosT_d = nc.dram_tensor("cosT", [128, S], bf16, kind="ExternalInput")
    sinT_d = nc.dram_tensor("sinT", [128, S], bf16, kind="ExternalInput")
    wp_d = nc.dram_tensor("wp", [HD, D], bf16, kind="ExternalInput")
    bp_d = nc.dram_tensor("bp", [128, 8], f32, kind="ExternalInput")
    mask01_d = nc.dram_tensor("mask01", [128, 128], bf16, kind="ExternalInput")
    ones64_d = nc.dram_tensor("ones64", [128, 64], bf16, kind="ExternalInput")
    ident_d = nc.dram_tensor("ident", [128, 128], bf16, kind="ExternalInput")
    if with_bias:
        bqkv_d = nc.dram_tensor("bqkv", [1, 768], bf16, kind="ExternalInput")
        onesrow_d = nc.dram_tensor("ones_row", [1, 512], bf16,
                                   kind="ExternalInput")
    outT_d = nc.dram_tensor("outT", [D, S], bf16, kind="ExternalOutput")

    with tile.TileContext(nc) as tc, ExitStack() as top:
        const = top.enter_context(tc.tile_pool(name="const", bufs=1))
        ident = const.tile([128, 128], bf16, tag="ident")
        nc.sync.dma_start(ident[:], ident_d.ap())
        mask01 = const.tile([128, 128], bf16, tag="mask01")
        nc.sync.dma_start(mask01[:], mask01_d.ap())
        bp_sb = const.tile([128, 8], f32, tag="bp")
        nc.sync.dma_start(bp_sb[:], bp_d.ap())
        if with_bias:
            bqkv_sb = const.tile([1, 768], bf16, tag="bqkv")
            nc.sync.dma_start(bqkv_sb[:], bqkv_d.ap())
            ones_row = const.tile([1, 512], bf16, tag="ones_row")
            nc.sync.dma_start(ones_row[:], onesrow_d.ap())

        persist = top.enter_context(tc.tile_pool(name="persist", bufs=1))
        qT = [persist.tile([128, S], bf16, tag=f"qT{hp}", name=f"qT{hp}")
              for hp in range(2)]
        kT = [persist.tile([128, S], bf16, tag=f"kT{hp}", name=f"kT{hp}")
              for hp in range(2)]
        v_sb = persist.tile([128, SB, 4, 65], bf16, tag="v")
        ones64 = const.tile([128, 64], bf16, tag="ones64")
        nc.sync.dma_start(ones64[:], ones64_d.ap())
        nc.scalar.copy(v_sb[:, :, :, 64],
                       ones64[:].rearrange("p (a b) -> p a b", a=SB))
        wp_sb = persist.tile([128, 2, D], bf16, tag="wp")
        for kc2 in range(2):
            nc.sync.dma_start(wp_sb[:, kc2, :],
                                wp_d.ap()[kc2 * 128:(kc2 + 1) * 128, :])
        attnT = [persist.tile([128, S], bf16, tag=f"attnT{hp}",
                              name=f"attnT{hp}") for hp in range(2)]
        cosT = persist.tile([128, S], bf16, tag="cosT")
        sinT = persist.tile([128, S], bf16, tag="sinT")
        for sg in range(4):
            sl = slice(sg * 512, (sg + 1) * 512)
            nc.sync.dma_start(cosT[:, sl], cosT_d.ap()[:, sl])
            nc.sync.dma_start(sinT[:, sl], sinT_d.ap()[:, sl])

        # ============ stage 1: QKV + rope (transposed layout) ============
        with ExitStack() as st1, nc.named_scope("qkv"):
            hT_pool = st1.enter_context(tc.tile_pool(name="hT", bufs=1))
            w_pool = st1.enter_context(tc.tile_pool(name="w", bufs=1))
            vT_pool = st1.enter_context(tc.tile_pool(name="vT", bufs=1))
            qkv_ps = st1.enter_context(
                tc.tile_pool(name="qkv_ps", bufs=3, space="PSUM"))
            tr_ps = st1.enter_context(
                tc.tile_pool(name="tr_ps", bufs=2, space="PSUM"))
            rope_pool = st1.enter_context(tc.tile_pool(name="rope", bufs=3))

            w_sb = [w_pool.tile([128, 768], bf16, tag=f"w{kc}", name=f"w{kc}")
                    for kc in range(KC)]
            hT_sb = [hT_pool.tile([128, S], bf16, tag=f"hT{kc}",
                                  name=f"hT{kc}") for kc in range(KC)]
            for kc in range(KC):
                nc.sync.dma_start(w_sb[kc][:],
                                  wqkv_d.ap()[kc * 128:(kc + 1) * 128, :])
                nc.sync.dma_start(hT_sb[kc][:, 0:512],
                                  hT_d.ap()[kc * 128:(kc + 1) * 128, 0:512])
            for sblk in range(1, NCH):
                sl = slice(sblk * 512, (sblk + 1) * 512)
                for kc in range(KC):
                    nc.sync.dma_start(hT_sb[kc][:, sl],
                                      hT_d.ap()[kc * 128:(kc + 1) * 128, sl])

            vT_sb = [vT_pool.tile([128, S], bf16, tag=f"vT{t}", name=f"vT{t}")
                     for t in range(2)]

            # v chunks first (do 4,5), then q/k: q hp0=0, k hp0=2, q hp1=1,
            # k hp1=3
            for do in (4, 5, 0, 2, 1, 3):
                for sblk in range(NCH):
                    sl = slice(sblk * 512, (sblk + 1) * 512)
                    qkv_p = qkv_ps.tile([128, 512], f32, tag="qkv_p")
                    for kc in range(KC):
                        nc.tensor.matmul(
                            qkv_p[:], w_sb[kc][:, do * 128:(do + 1) * 128],
                            hT_sb[kc][:, sl], start=(kc == 0),
                            stop=(kc == KC - 1 and not with_bias))
                    if with_bias:
                        nc.tensor.matmul(
                            qkv_p[:],
                            bqkv_sb[:, do * 128:(do + 1) * 128],
                            ones_row[:], start=False, stop=True)
                    if do >= 4:
                        nc.scalar.copy(vT_sb[do - 4][:, sl], qkv_p[:])
                    else:
                        dest = (qT if do in (0, 1) else kT)[do % 2]
                        qraw = rope_pool.tile([128, 512], bf16, tag="qraw")
                        nc.vector.tensor_copy(qraw[:], qkv_p[:])
                        qsw = rope_pool.tile([128, 512], bf16, tag="qsw")
                        for blk in range(4):
                            src = (blk * 32 + 32) % 64 + 64 * (blk // 2)
                            nc.sync.dma_start(
                                qsw[blk * 32:blk * 32 + 32, :],
                                qraw[src:src + 32, :])
                        qcos = rope_pool.tile([128, 512], bf16, tag="qcos")
                        nc.vector.tensor_tensor(qcos[:], qkv_p[:],
                                                cosT[:, sl], op=ALU.mult)
                        qsin = rope_pool.tile([128, 512], bf16, tag="qsin")
                        nc.vector.tensor_tensor(qsin[:], qsw[:],
                                                sinT[:, sl], op=ALU.mult)
                        nc.vector.tensor_tensor(dest[:, sl], qcos[:],
                                                qsin[:], op=ALU.add)
                # after v chunks: transpose vT -> v natural layout
                if do == 5:
                    for t in range(2):
                        for sb in range(SB):
                            tp = tr_ps.tile([128, 128], bf16, tag="tp")
                            nc.tensor.matmul(
                                tp[:], vT_sb[t][:, sb * 128:(sb + 1) * 128],
                                ident[:], is_transpose=True,
                                start=True, stop=True)
                            nc.vector.tensor_copy(
                                v_sb[:, sb, 2 * t:2 * t + 2, 0:64],
                                tp[:].rearrange("p (h d) -> p h d", h=2))

        # ============ stages 2+3 interleaved ============
        st23 = top.enter_context(ExitStack())
        st_ps = st23.enter_context(
            tc.tile_pool(name="st_ps", bufs=2, space="PSUM"))
        out_ps = st23.enter_context(
            tc.tile_pool(name="out_ps", bufs=2, space="PSUM"))
        pj_ps = st23.enter_context(
            tc.tile_pool(name="pj_ps", bufs=2, space="PSUM"))
        pt_pool = st23.enter_context(tc.tile_pool(name="pt", bufs=17))
        u_pool = st23.enter_context(tc.tile_pool(name="u", bufs=3))
        nrm_pool = st23.enter_context(tc.tile_pool(name="nrm", bufs=3))
        pj_sb = st23.enter_context(tc.tile_pool(name="pj_sb", bufs=3))

        def attn_chunk(c, hp):
            nkb = 4 * c + 4
            pts = []
            for kb in range(nkb):
                q0 = max(512 * c, 128 * kb)
                off = q0 - 512 * c
                st_p = st_ps.tile([128, 2, 512], f32, tag="st_p")
                for h2 in range(2):
                    nc.tensor.matmul(
                        st_p[:, h2, off:512],
                        kT[hp][h2 * 64:(h2 + 1) * 64,
                               kb * 128:(kb + 1) * 128],
                        qT[hp][h2 * 64:(h2 + 1) * 64, q0:512 * (c + 1)],
                        start=True, stop=True, tile_position=(h2 * 64, 0))
                pt = pt_pool.tile([128, 2, 512], bf16, tag="pt")
                nc.scalar.activation(pt[:, :, off:512], st_p[:, :, off:512],
                                     AF.Exp, scale=0.125)
                if 128 * kb >= 512 * c:
                    for h2 in range(2):
                        nc.gpsimd.tensor_mul(pt[:, h2, off:off + 128],
                                             pt[:, h2, off:off + 128],
                                             mask01[:])
                pts.append((kb, off, pt))

            for h2 in range(2):
                h = 2 * hp + h2
                o_p = out_ps.tile([128, 512], f32, tag="o_p")
                for (kb, off, pt) in pts:
                    nc.tensor.matmul(
                        o_p[0:65, off:512], v_sb[:, kb, h, :],
                        pt[:, h2, off:512],
                        start=(kb == 0), stop=(kb == nkb - 1))
                u = u_pool.tile([65, 512], f32, tag="u")
                nc.vector.tensor_copy(u[:], o_p[0:65, :])
                den0 = nrm_pool.tile([1, 512], f32, tag="den0")
                nc.sync.dma_start(den0[:], u[64:65, :])
                rcp0 = nrm_pool.tile([1, 512], f32, tag="rcp0")
                nc.vector.reciprocal_approx_fast(rcp0[:], den0[:])
                bc = nrm_pool.tile([64, 512], f32, tag="bc")
                nc.gpsimd.partition_broadcast(bc[:], rcp0[:])
                csl = slice(c * 512, (c + 1) * 512)
                if h2 == 0:
                    nc.vector.tensor_tensor(attnT[hp][0:64, csl],
                                            u[0:64, :], bc[:], op=ALU.mult)
                else:
                    aTo = u_pool.tile([64, 512], bf16, tag="aTo")
                    nc.vector.tensor_tensor(aTo[:], u[0:64, :], bc[:],
                                            op=ALU.mult)
                    nc.sync.dma_start(attnT[hp][64:128, csl], aTo[:])

        def proj_chunk(c):
            csl = slice(c * 512, (c + 1) * 512)
            for dd in range(8):
                pp = pj_ps.tile([128, 512], f32, tag="pp")
                for kc2 in range(2):
                    nc.tensor.matmul(
                        pp[:], wp_sb[:, kc2, dd * 128:(dd + 1) * 128],
                        attnT[kc2][:, csl],
                        start=(kc2 == 0), stop=(kc2 == 1))
                po = pj_sb.tile([128, 512], bf16, tag="po")
                nc.scalar.activation(po[:], pp[:], AF.Identity,
                                     bias=bp_sb[:, dd:dd + 1])
                nc.sync.dma_start(
                    outT_d.ap()[dd * 128:(dd + 1) * 128, csl], po[:])

        with nc.named_scope("attn"):
            attn_chunk(0, 0)
            attn_chunk(0, 1)
            attn_chunk(1, 0)
            proj_chunk(0)
            attn_chunk(1, 1)
            attn_chunk(2, 0)
            proj_chunk(1)
            attn_chunk(2, 1)
            attn_chunk(3, 0)
            proj_chunk(2)
            attn_chunk(3, 1)
            proj_chunk(3)

    nc.finalize()
    return nc


def make_core_inputs(inputs, core, with_bias, _cache={}):
    """Host-side shard prep for one core."""
    b, g = core // 4, core % 4
    key = id(inputs)
    if _cache.get("key") != key:
        _cache.clear()
        _cache["key"] = key

    if ("hT", b) not in _cache:
        hidden = np.asarray(inputs["hidden_states"], dtype=np.float32)
        _cache[("hT", b)] = np.ascontiguousarray(hidden[b].T).astype(BF)
    if ("trig", b) not in _cache:
        pos = np.asarray(inputs["position_ids"])
        inv_freq = (1.0 / (10000.0 **
                           (np.arange(0, 64, 2, dtype=np.float64) / 64.0)))
        # pattern[d, s] = pos[s] * invf[d % 32] over d in [0, 64)
        freqsT = inv_freq[:, None] * pos[b].astype(np.float64)[None, :]
        embT = np.concatenate([freqsT, freqsT], axis=0)     # [64, S]
        cosp = np.cos(embT)
        sinp = np.sin(embT)
        sinp[:32, :] *= -1.0
        _cache[("trig", b)] = (np.tile(cosp, (2, 1)).astype(BF),
                               np.tile(sinp, (2, 1)).astype(BF))

    caw = np.asarray(inputs["c_attn_w"], dtype=np.float32)
    cab = np.asarray(inputs["c_attn_b"], dtype=np.float32)
    cpw = np.asarray(inputs["c_proj_w"], dtype=np.float32)
    cpb = np.asarray(inputs["c_proj_b"], dtype=np.float32)

    cs = slice(g * HD, (g + 1) * HD)
    wqkv = np.concatenate(
        [caw[:, cs], caw[:, D + g * HD:D + (g + 1) * HD],
         caw[:, 2 * D + g * HD:2 * D + (g + 1) * HD]], axis=1)

    bp = (cpb if g == 0 else np.zeros_like(cpb)).reshape(8, 128).T.copy()

    r = np.arange(128)
    mask01 = (r[None, :] >= r[:, None]).astype(BF)
    cosT, sinT = _cache[("trig", b)]

    out = {
        "hT": _cache[("hT", b)],
        "wqkv": np.ascontiguousarray(wqkv).astype(BF),
        "cosT": cosT,
        "sinT": sinT,
        "wp": np.ascontiguousarray(cpw[cs, :]).astype(BF),
        "bp": np.ascontiguousarray(bp.astype(np.float32)),
        "mask01": mask01,
        "ones64": np.ones((128, 64), BF),
        "ident": np.eye(128).astype(BF),
    }
    if with_bias:
        bqkv = np.concatenate(
            [cab[cs], cab[D + g * HD:D + (g + 1) * HD],
             cab[2 * D + g * HD:2 * D + (g + 1) * HD]])[None, :]
        out["bqkv"] = bqkv.astype(BF)
        out["ones_row"] = np.ones((1, 512), BF)
    return out


_NC_CACHE = {}


def run(inputs, trace=False, **spmd_kwargs):
    """Shard, execute on 8 cores, unshard. Returns (output, BassKernelResults)."""
    with_bias = bool(np.any(np.asarray(inputs["c_attn_b"])))
    if with_bias not in _NC_CACHE:
        _NC_CACHE[with_bias] = build_attention_nc(with_bias=with_bias,
                                                  num_devices=8)
    nc = _NC_CACHE[with_bias]
    in_maps = [make_core_inputs(inputs, c, with_bias) for c in range(8)]
    res = run_bass_kernel_spmd(nc, in_maps, core_ids=list(range(8)),
                               trace=trace, **spmd_kwargs)
    outs = []
    for b in range(2):
        acc = np.zeros((D, S), np.float32)
        for g in range(4):
            acc += res.results[b * 4 + g]["outT"].astype(np.float32)
        outs.append(acc.T)
    return np.stack(outs, axis=0), res


def kernel(**inputs) -> np.ndarray:
    out, _ = run(inputs, trace=False)
    return out
